# revision 6
# baseline (speedup 1.0000x reference)
"""Trainium2 Bass kernel for nn_Attention2D: 2D attention over spatial axis.

Reference computation (per batch element b):
  qkv = w_qkv @ x          (1x1 conv == channel GEMM), x: [256, 4096]
  q,k,v: [8 heads, 64, 4096];  q *= 64**-0.5
  sim[h,i,j] = sum_n q[h,i,n] k[h,j,n]   (contraction over SPATIAL n=4096)
  attn = softmax(sim, axis=j)
  out[h,i,n] = sum_j attn[h,i,j] v[h,j,n]
  y = w_out @ out + b_out

Sharding: data-parallel over batch, 16 elems / 8 cores = 2 per core.

Algebraic restructuring (the attention contracts over n, so everything
factors through the 256x256 Gram matrix):
  G    = X @ X.T                      [256,256]   (537 MF)
  sim_h = Wq_h @ G @ Wk_h.T           via GqT = G @ Wq.T then tiny MMs
  attn  = softmax(sim)                (unnormalized exp; 1/Z folded later)
  M    = sum_h Wout_h @ attn_h @ Wv_h [256,256]   (tiny head-space GEMMs)
  y    = M @ X + b                    (537 MF)
This is ~4x fewer FLOPs than materializing q,k,v [512,4096].

Device dataflow per batch element (fp16 matmuls, fp32 PSUM):
  - xT [4096,256] built by 64 DMA transposes straight from DRAM.
  - G: xT-stationary MMs accumulated over 32 n-chunks (2 row tiles).
  - GqT = G @ WqT (G symmetric, so it is its own lhsT).
  - sim per head-pair: 2 N=128 MMs (both heads packed in col groups).
  - softmax over free dim j; unnormalized exp written block-diagonally,
    one DMA transpose gives attnT per pair.
  - AWv_h = attn_h @ Wv_h via packed row+col diagonal MMs; 1/Z applied
    per-partition in the psum->sbuf copy.
  - MT = (Wout @ AWv).T = AWv.T @ WoutT (gives M in lhsT layout directly).
  - y = M @ X via MT-stationary MMs against x tiles + per-partition bias.
"""
import numpy as np

HEADS = 8
DH = 64
DIM = 256
HIDDEN = 512
B = 16
N = 4096            # h*w = 64*64
N_CORES = 8
B_PER_CORE = B // N_CORES
NT = N // 512       # 8 moving tiles of 512
NTP = NT // 2       # 4 double-tiles of 1024
NCH = N // 128      # 32 n-chunks of 128
PAIRS = HEADS // 2  # 4 head pairs
CC = DIM // 128     # 2 channel chunks
KC = HIDDEN // 128  # 4 hidden chunks

_nc_cache = {}


def _build():
    if "nc" in _nc_cache:
        return _nc_cache["nc"]
    from contextlib import ExitStack
    import concourse.bacc as bacc
    import concourse.tile as tile
    from concourse import mybir

    f16 = mybir.dt.float16
    f32 = mybir.dt.float32
    Exp = mybir.ActivationFunctionType.Exp
    X = mybir.AxisListType.X

    nc = bacc.Bacc("TRN2", target_bir_lowering=False, debug=False,
                   num_devices=N_CORES)
    x_d = nc.dram_tensor("x", [B_PER_CORE, DIM, N], f16, kind="ExternalInput").ap()
    wqk_d = nc.dram_tensor("wqk", [DIM, 2 * HIDDEN], f16, kind="ExternalInput").ap()
    wvn_d = nc.dram_tensor("wvn", [HIDDEN, DIM], f16, kind="ExternalInput").ap()
    wout_d = nc.dram_tensor("wout", [HIDDEN, DIM], f16, kind="ExternalInput").ap()
    b_d = nc.dram_tensor("b", [DIM], f32, kind="ExternalInput").ap()
    y_d = nc.dram_tensor("y", [B_PER_CORE, DIM, N], f32, kind="ExternalOutput").ap()

    with tile.TileContext(nc) as tc, ExitStack() as ctx:
        consts = ctx.enter_context(tc.tile_pool(name="consts", bufs=1))
        xp = ctx.enter_context(tc.tile_pool(name="xp", bufs=2))
        xtp = ctx.enter_context(tc.tile_pool(name="xtp", bufs=2))
        midp = ctx.enter_context(tc.tile_pool(name="midp", bufs=2))
        smallp = ctx.enter_context(tc.tile_pool(name="smallp", bufs=4))
        stagep = ctx.enter_context(tc.tile_pool(name="stagep", bufs=4))
        pb = ctx.enter_context(tc.tile_pool(name="pb", bufs=2, space="PSUM"))
        psm = ctx.enter_context(tc.tile_pool(name="psm", bufs=4, space="PSUM"))

        # ---- weights (loaded once) ----
        wqk_t = consts.tile([128, CC, 2 * HIDDEN], f16)
        nc.sync.dma_start(out=wqk_t[:], in_=wqk_d.rearrange("(c p) o -> p c o", p=128))
        wvn_t = consts.tile([128, PAIRS, DIM], f16)
        nc.sync.dma_start(out=wvn_t[:], in_=wvn_d.rearrange("(k p) o -> p k o", p=128))
        wout_t = consts.tile([128, KC, DIM], f16)
        nc.sync.dma_start(out=wout_t[:], in_=wout_d.rearrange("(k p) o -> p k o", p=128))
        b_t = consts.tile([128, 2], f32)
        nc.sync.dma_start(out=b_t[:], in_=b_d.rearrange("(m p) -> p m", p=128))

        for e in range(B_PER_CORE):
            # ---- xT via DMA transposes straight from DRAM ----
            xT_t = xtp.tile([128, NCH, DIM], f16, tag="xT")
            for t in range(NCH):
                for c in range(CC):
                    nc.sync.dma_start_transpose(
                        out=xT_t[:, t, c * 128:(c + 1) * 128],
                        in_=x_d[e, c * 128:(c + 1) * 128, t * 128:(t + 1) * 128])

            # ---- x tiles (for the final y = M @ X GEMM) ----
            x_t = xp.tile([128, CC, N], f16, tag="x")
            x_src = x_d[e].rearrange("(c p) n -> p c n", p=128)
            for g in range(4):
                nc.sync.dma_start(out=x_t[:, :, g * 1024:(g + 1) * 1024],
                                  in_=x_src[:, :, g * 1024:(g + 1) * 1024])

            # ---- G = X @ X.T  [256, 256] ----
            ps_g = [psm.tile([128, DIM], f32, tag="psm", name=f"ps_g{i}") for i in range(2)]
            for t in range(NCH):
                for m in range(2):
                    nc.tensor.matmul(ps_g[m][:], xT_t[:, t, m * 128:(m + 1) * 128],
                                     xT_t[:, t, :], start=(t == 0), stop=(t == NCH - 1))
            g_t = midp.tile([128, 2, DIM], f16, tag="g")
            nc.vector.tensor_copy(g_t[:, 0, :], ps_g[0][:])
            nc.scalar.copy(g_t[:, 1, :], ps_g[1][:])

            # ---- GqT = G @ WqT  [256, 512] (G symmetric -> G is its own lhsT) ----
            ps_gq = pb.tile([128, 1024], f32, tag="pb")
            for m in range(2):
                for c in range(CC):
                    nc.tensor.matmul(ps_gq[:, m * 512:(m + 1) * 512],
                                     g_t[:, c, m * 128:(m + 1) * 128],
                                     wqk_t[:, c, 0:HIDDEN],
                                     start=(c == 0), stop=(c == CC - 1))
            gq_t = midp.tile([128, 2, HIDDEN], f16, tag="gq")
            nc.vector.tensor_copy(gq_t[:], ps_gq[:].rearrange("p (c o) -> p c o", c=2))

            # ---- per pair: sim + softmax + transpose ----
            attnTs = []
            rzs = []
            for p in range(PAIRS):
                ps_s = psm.tile([128, 128], f32, tag="psm")
                co = p * 128
                for c in range(CC):
                    nc.tensor.matmul(ps_s[:], gq_t[:, c, co:co + 128],
                                     wqk_t[:, c, HIDDEN + co:HIDDEN + co + 128],
                                     start=(c == 0), stop=(c == CC - 1))
                negmax = smallp.tile([128, 1], f32, tag="negmax")
                nc.vector.reduce_max(negmax[0:64, :], ps_s[0:64, 0:64],
                                     axis=X, negate=True)
                nc.vector.reduce_max(negmax[64:128, :], ps_s[64:128, 64:128],
                                     axis=X, negate=True)
                esum = smallp.tile([128, 1], f32, tag="esum")
                attn_pad = smallp.tile([128, 128], f16, tag="attn_pad")
                nc.scalar.activation(attn_pad[0:64, 0:64], ps_s[0:64, 0:64], Exp,
                                     bias=negmax[0:64, :], accum_out=esum[0:64, :])
                nc.scalar.activation(attn_pad[64:128, 64:128], ps_s[64:128, 64:128],
                                     Exp, bias=negmax[64:128, :],
                                     accum_out=esum[64:128, :])
                rz = smallp.tile([128, 1], f32, tag="rz")
                nc.vector.reciprocal(rz[:], esum[:])
                attnT = smallp.tile([128, 128], f16, tag="attnT")
                nc.sync.dma_start_transpose(out=attnT[:], in_=attn_pad[:])
                attnTs.append(attnT)
                rzs.append(rz)

            # ---- AWv_h = attn_h @ Wv_h (unnormalized; 1/Z in the copy) ----
            awv_t = midp.tile([128, KC, DIM], f16, tag="awv")
            for p in range(PAIRS):
                ps_a = psm.tile([128, DIM], f32, tag="psm")
                nc.tensor.matmul(ps_a[0:64, :], attnTs[p][0:64, 0:64],
                                 wvn_t[0:64, p, :], start=True, stop=True)
                nc.tensor.matmul(ps_a[64:128, :], attnTs[p][64:128, 64:128],
                                 wvn_t[64:128, p, :], start=True, stop=True)
                if p % 2 == 0:
                    nc.vector.tensor_scalar_mul(awv_t[:, p, :], ps_a[:], rzs[p][:])
                else:
                    nc.scalar.mul(awv_t[:, p, :], ps_a[:], rzs[p][:])

            # ---- MT = AWv.T @ WoutT  (= M in lhsT layout) [256, 256] ----
            ps_m = [psm.tile([128, DIM], f32, tag="psm", name=f"ps_m{i}") for i in range(2)]
            for k in range(KC):
                for m in range(2):
                    nc.tensor.matmul(ps_m[m][:], awv_t[:, k, m * 128:(m + 1) * 128],
                                     wout_t[:, k, :], start=(k == 0), stop=(k == KC - 1))
            mt_t = midp.tile([128, 2, DIM], f16, tag="mt")
            nc.vector.tensor_copy(mt_t[:, 0, :], ps_m[0][:])
            nc.scalar.copy(mt_t[:, 1, :], ps_m[1][:])

            # ---- y = M @ X + b ----
            for m2 in range(2):
                for tp in range(NTP):
                    ps_y = pb.tile([128, 1024], f32, tag="pb")
                    for c in range(CC):
                        w = mt_t[:, c, m2 * 128:(m2 + 1) * 128]
                        nc.tensor.matmul(ps_y[:, 0:512], w,
                                         x_t[:, c, tp * 1024:tp * 1024 + 512],
                                         start=(c == 0), stop=(c == CC - 1))
                        nc.tensor.matmul(ps_y[:, 512:1024], w,
                                         x_t[:, c, tp * 1024 + 512:(tp + 1) * 1024],
                                         start=(c == 0), stop=(c == CC - 1))
                    y_stage = stagep.tile([128, 1024], f32, tag="y_stage")
                    if tp % 2 == 0:
                        nc.vector.tensor_scalar_add(y_stage[:], ps_y[:],
                                                    b_t[:, m2:m2 + 1])
                    else:
                        nc.scalar.add(y_stage[:], ps_y[:], b_t[:, m2:m2 + 1])
                    nc.sync.dma_start(
                        out=y_d[e, m2 * 128:(m2 + 1) * 128, tp * 1024:(tp + 1) * 1024],
                        in_=y_stage[:])

    nc.compile()
    _nc_cache["nc"] = nc
    return nc


def _prep_inputs(x, w_qkv, w_out, b_out):
    scale = DH ** (-0.5)
    wq = (w_qkv[0:HIDDEN] * scale).astype(np.float16)       # [512, 256]
    wk = w_qkv[HIDDEN:2 * HIDDEN].astype(np.float16)
    wv_nat = w_qkv[2 * HIDDEN:3 * HIDDEN].astype(np.float16).copy()  # [512, 256]
    wqk = np.concatenate([wq.T, wk.T], axis=1).copy()       # [256, 1024]
    wout_T = w_out.T.astype(np.float16).copy()              # [512, 256]
    b = b_out.astype(np.float32)
    x16 = np.ascontiguousarray(x.reshape(B, DIM, N)).astype(np.float16)
    return x16, wqk, wv_nat, wout_T, b


def _run(x, w_qkv, w_out, b_out, trace=False, tmpdir=None):
    from concourse.bass_utils import run_bass_kernel_spmd

    nc = _build()
    x16, wqk, wv_nat, wout_T, b = _prep_inputs(x, w_qkv, w_out, b_out)
    in_maps = [
        {"x": x16[i * B_PER_CORE:(i + 1) * B_PER_CORE], "wqk": wqk, "wvn": wv_nat,
         "wout": wout_T, "b": b}
        for i in range(N_CORES)
    ]
    kw = {}
    if trace:
        kw = {"trace": True, "tmpdir": tmpdir}
    res = run_bass_kernel_spmd(nc, in_maps, core_ids=list(range(N_CORES)), **kw)
    y = np.concatenate([res.results[i]["y"] for i in range(N_CORES)], axis=0)
    return y.reshape(B, DIM, 64, 64), res


def kernel(x, w_qkv, w_out, b_out):
    y, _ = _run(np.asarray(x), np.asarray(w_qkv), np.asarray(w_out),
                np.asarray(b_out))
    return y


# revision 8
# speedup vs baseline: 2.6256x; 2.6256x over previous
"""Trainium2 Bass kernel for nn_Attention2D: 2D attention over spatial axis.

Reference computation (per batch element b):
  qkv = w_qkv @ x          (1x1 conv == channel GEMM), x: [256, 4096]
  q,k,v: [8 heads, 64, 4096];  q *= 64**-0.5
  sim[h,i,j] = sum_n q[h,i,n] k[h,j,n]   (contraction over SPATIAL n=4096)
  attn = softmax(sim, axis=j)
  out[h,i,n] = sum_j attn[h,i,j] v[h,j,n]
  y = w_out @ out + b_out

Sharding: data-parallel over batch, 16 elems / 8 cores = 2 per core.

Algebraic restructuring (the attention contracts over n, so everything
factors through the 256x256 Gram matrix):
  G    = X @ X.T                      [256,256]   (537 MF)
  sim_h = Wq_h @ G @ Wk_h.T           via GqT = G @ Wq.T then tiny MMs
  attn  = softmax(sim)                (unnormalized exp; 1/Z folded later)
  M    = sum_h Wout_h @ attn_h @ Wv_h [256,256]   (tiny head-space GEMMs)
  y    = M @ X + b                    (537 MF)
This is ~4x fewer FLOPs than materializing q,k,v [512,4096].

Device dataflow per batch element (fp16 matmuls, fp32 PSUM):
  - xT [4096,256] built by 64 DMA transposes straight from DRAM.
  - G: xT-stationary MMs accumulated over 32 n-chunks (2 row tiles).
  - GqT = G @ WqT (G symmetric, so it is its own lhsT).
  - sim per head-pair: 2 N=128 MMs (both heads packed in col groups).
  - softmax over free dim j; unnormalized exp written block-diagonally,
    one DMA transpose gives attnT per pair.
  - AWv_h = attn_h @ Wv_h via packed row+col diagonal MMs; 1/Z applied
    per-partition in the psum->sbuf copy.
  - MT = (Wout @ AWv).T = AWv.T @ WoutT (gives M in lhsT layout directly).
  - y = M @ X via MT-stationary MMs against x tiles + per-partition bias.
"""
import numpy as np

HEADS = 8
DH = 64
DIM = 256
HIDDEN = 512
B = 16
N = 4096            # h*w = 64*64
N_CORES = 8
B_PER_CORE = B // N_CORES
NT = N // 512       # 8 moving tiles of 512
NTP = NT // 2       # 4 double-tiles of 1024
NCH = N // 128      # 32 n-chunks of 128
PAIRS = HEADS // 2  # 4 head pairs
CC = DIM // 128     # 2 channel chunks
KC = HIDDEN // 128  # 4 hidden chunks

_nc_cache = {}


def _build():
    if "nc" in _nc_cache:
        return _nc_cache["nc"]
    from contextlib import ExitStack
    import concourse.bacc as bacc
    import concourse.tile as tile
    from concourse import mybir

    f16 = mybir.dt.float16
    f32 = mybir.dt.float32
    Exp = mybir.ActivationFunctionType.Exp
    X = mybir.AxisListType.X

    nc = bacc.Bacc("TRN2", target_bir_lowering=False, debug=False,
                   num_devices=N_CORES)
    x_d = nc.dram_tensor("x", [B_PER_CORE, DIM, N], f16, kind="ExternalInput").ap()
    wqk_d = nc.dram_tensor("wqk", [DIM, 2 * HIDDEN], f16, kind="ExternalInput").ap()
    wvn_d = nc.dram_tensor("wvn", [HIDDEN, DIM], f16, kind="ExternalInput").ap()
    wout_d = nc.dram_tensor("wout", [HIDDEN, DIM], f16, kind="ExternalInput").ap()
    b_d = nc.dram_tensor("b", [DIM], f32, kind="ExternalInput").ap()
    id_d = nc.dram_tensor("ident", [128, 128], f16, kind="ExternalInput").ap()
    y_d = nc.dram_tensor("y", [B_PER_CORE, DIM, N], f32, kind="ExternalOutput").ap()

    with tile.TileContext(nc) as tc, ExitStack() as ctx:
        consts = ctx.enter_context(tc.tile_pool(name="consts", bufs=1))
        xp = ctx.enter_context(tc.tile_pool(name="xp", bufs=2))
        xtp = ctx.enter_context(tc.tile_pool(name="xtp", bufs=2))
        midp = ctx.enter_context(tc.tile_pool(name="midp", bufs=2))
        smallp = ctx.enter_context(tc.tile_pool(name="smallp", bufs=4))
        stagep = ctx.enter_context(tc.tile_pool(name="stagep", bufs=4))
        pb = ctx.enter_context(tc.tile_pool(name="pb", bufs=2, space="PSUM"))
        psm = ctx.enter_context(tc.tile_pool(name="psm", bufs=4, space="PSUM"))

        # ---- weights (loaded once) ----
        wqk_t = consts.tile([128, CC, 2 * HIDDEN], f16)
        nc.sync.dma_start(out=wqk_t[:], in_=wqk_d.rearrange("(c p) o -> p c o", p=128))
        wvn_t = consts.tile([128, PAIRS, DIM], f16)
        nc.sync.dma_start(out=wvn_t[:], in_=wvn_d.rearrange("(k p) o -> p k o", p=128))
        wout_t = consts.tile([128, KC, DIM], f16)
        nc.sync.dma_start(out=wout_t[:], in_=wout_d.rearrange("(k p) o -> p k o", p=128))
        b_t = consts.tile([128, 2], f32)
        nc.sync.dma_start(out=b_t[:], in_=b_d.rearrange("(m p) -> p m", p=128))
        id_t = consts.tile([128, 128], f16)
        nc.sync.dma_start(out=id_t[:], in_=id_d)

        for e in range(B_PER_CORE):
            # ---- xT via ONE whole-x DMA transpose (layout n = t*128 + p) ----
            xT_t = xtp.tile([128, NCH, DIM], f16, tag="xT")
            tq = nc.sync if e % 2 == 0 else nc.scalar
            tq.dma_start_transpose(out=xT_t[:], in_=x_d[e])

            # ---- x tiles (for the final y = M @ X GEMM) ----
            x_t = xp.tile([128, CC, N], f16, tag="x")
            x_src = x_d[e].rearrange("(c p) n -> p c n", p=128)
            for g in range(4):
                nc.sync.dma_start(out=x_t[:, :, g * 1024:(g + 1) * 1024],
                                  in_=x_src[:, :, g * 1024:(g + 1) * 1024])

            # ---- G = X @ X.T  [256, 256] ----
            ps_g = [psm.tile([128, DIM], f32, tag="psm", name=f"ps_g{i}") for i in range(2)]
            for t in range(NCH):
                for m in range(2):
                    nc.tensor.matmul(ps_g[m][:], xT_t[:, t, m * 128:(m + 1) * 128],
                                     xT_t[:, t, :], start=(t == 0), stop=(t == NCH - 1))
            g_t = midp.tile([128, 2, DIM], f16, tag="g")
            nc.vector.tensor_copy(g_t[:, 0, :], ps_g[0][:])
            nc.scalar.copy(g_t[:, 1, :], ps_g[1][:])

            # ---- GqT = G @ WqT  [256, 512] (G symmetric -> G is its own lhsT) ----
            ps_gq = pb.tile([128, 1024], f32, tag="pb")
            for m in range(2):
                for c in range(CC):
                    nc.tensor.matmul(ps_gq[:, m * 512:(m + 1) * 512],
                                     g_t[:, c, m * 128:(m + 1) * 128],
                                     wqk_t[:, c, 0:HIDDEN],
                                     start=(c == 0), stop=(c == CC - 1))
            gq_t = midp.tile([128, 2, HIDDEN], f16, tag="gq")
            nc.vector.tensor_copy(gq_t[:], ps_gq[:].rearrange("p (c o) -> p c o", c=2))

            # ---- per pair: sim + softmax + transpose ----
            attnTs = []
            rzs = []
            for p in range(PAIRS):
                ps_s = psm.tile([128, 128], f32, tag="psm")
                co = p * 128
                for c in range(CC):
                    nc.tensor.matmul(ps_s[:], gq_t[:, c, co:co + 128],
                                     wqk_t[:, c, HIDDEN + co:HIDDEN + co + 128],
                                     start=(c == 0), stop=(c == CC - 1))
                negmax = smallp.tile([128, 1], f32, tag="negmax")
                nc.vector.reduce_max(negmax[0:64, :], ps_s[0:64, 0:64],
                                     axis=X, negate=True)
                nc.vector.reduce_max(negmax[64:128, :], ps_s[64:128, 64:128],
                                     axis=X, negate=True)
                esum = smallp.tile([128, 1], f32, tag="esum")
                attn_pad = smallp.tile([128, 128], f16, tag="attn_pad")
                nc.gpsimd.memset(attn_pad[0:64, 64:128], 0.0)
                nc.gpsimd.memset(attn_pad[64:128, 0:64], 0.0)
                nc.scalar.activation(attn_pad[0:64, 0:64], ps_s[0:64, 0:64], Exp,
                                     bias=negmax[0:64, :], accum_out=esum[0:64, :])
                nc.scalar.activation(attn_pad[64:128, 64:128], ps_s[64:128, 64:128],
                                     Exp, bias=negmax[64:128, :],
                                     accum_out=esum[64:128, :])
                rz = smallp.tile([128, 1], f32, tag="rz")
                nc.vector.reciprocal(rz[:], esum[:])
                ps_t = psm.tile([128, 128], f32, tag="psm", name="ps_t")
                nc.tensor.matmul(ps_t[:], attn_pad[:], id_t[:], start=True, stop=True)
                attnT = smallp.tile([128, 128], f16, tag="attnT")
                if p % 2 == 0:
                    nc.vector.tensor_copy(attnT[:], ps_t[:])
                else:
                    nc.scalar.copy(attnT[:], ps_t[:])
                attnTs.append(attnT)
                rzs.append(rz)

            # ---- AWv_h = attn_h @ Wv_h (unnormalized; 1/Z in the copy) ----
            awv_t = midp.tile([128, KC, DIM], f16, tag="awv")
            for p in range(PAIRS):
                ps_a = psm.tile([128, DIM], f32, tag="psm")
                nc.tensor.matmul(ps_a[0:64, :], attnTs[p][0:64, 0:64],
                                 wvn_t[0:64, p, :], start=True, stop=True)
                nc.tensor.matmul(ps_a[64:128, :], attnTs[p][64:128, 64:128],
                                 wvn_t[64:128, p, :], start=True, stop=True)
                if p % 2 == 0:
                    nc.vector.tensor_scalar_mul(awv_t[:, p, :], ps_a[:], rzs[p][:])
                else:
                    nc.scalar.mul(awv_t[:, p, :], ps_a[:], rzs[p][:])

            # ---- MT = AWv.T @ WoutT  (= M in lhsT layout) [256, 256] ----
            ps_m = [psm.tile([128, DIM], f32, tag="psm", name=f"ps_m{i}") for i in range(2)]
            for k in range(KC):
                for m in range(2):
                    nc.tensor.matmul(ps_m[m][:], awv_t[:, k, m * 128:(m + 1) * 128],
                                     wout_t[:, k, :], start=(k == 0), stop=(k == KC - 1))
            mt_t = midp.tile([128, 2, DIM], f16, tag="mt")
            nc.vector.tensor_copy(mt_t[:, 0, :], ps_m[0][:])
            nc.scalar.copy(mt_t[:, 1, :], ps_m[1][:])

            # ---- y = M @ X + b ----
            for m2 in range(2):
                for tp in range(NTP):
                    ps_y = pb.tile([128, 1024], f32, tag="pb")
                    for c in range(CC):
                        w = mt_t[:, c, m2 * 128:(m2 + 1) * 128]
                        nc.tensor.matmul(ps_y[:, 0:512], w,
                                         x_t[:, c, tp * 1024:tp * 1024 + 512],
                                         start=(c == 0), stop=(c == CC - 1))
                        nc.tensor.matmul(ps_y[:, 512:1024], w,
                                         x_t[:, c, tp * 1024 + 512:(tp + 1) * 1024],
                                         start=(c == 0), stop=(c == CC - 1))
                    y_stage = stagep.tile([128, 1024], f32, tag="y_stage")
                    if tp % 2 == 0:
                        nc.vector.tensor_scalar_add(y_stage[:], ps_y[:],
                                                    b_t[:, m2:m2 + 1])
                    else:
                        nc.scalar.add(y_stage[:], ps_y[:], b_t[:, m2:m2 + 1])
                    nc.sync.dma_start(
                        out=y_d[e, m2 * 128:(m2 + 1) * 128, tp * 1024:(tp + 1) * 1024],
                        in_=y_stage[:])

    nc.compile()
    _nc_cache["nc"] = nc
    return nc


def _prep_inputs(x, w_qkv, w_out, b_out):
    scale = DH ** (-0.5)
    wq = (w_qkv[0:HIDDEN] * scale).astype(np.float16)       # [512, 256]
    wk = w_qkv[HIDDEN:2 * HIDDEN].astype(np.float16)
    wv_nat = w_qkv[2 * HIDDEN:3 * HIDDEN].astype(np.float16).copy()  # [512, 256]
    wqk = np.concatenate([wq.T, wk.T], axis=1).copy()       # [256, 1024]
    wout_T = w_out.T.astype(np.float16).copy()              # [512, 256]
    b = b_out.astype(np.float32)
    x16 = np.ascontiguousarray(x.reshape(B, DIM, N)).astype(np.float16)
    ident = np.eye(128, dtype=np.float16)
    return x16, wqk, wv_nat, wout_T, b, ident


def _run(x, w_qkv, w_out, b_out, trace=False, tmpdir=None):
    from concourse.bass_utils import run_bass_kernel_spmd

    nc = _build()
    x16, wqk, wv_nat, wout_T, b, ident = _prep_inputs(x, w_qkv, w_out, b_out)
    in_maps = [
        {"x": x16[i * B_PER_CORE:(i + 1) * B_PER_CORE], "wqk": wqk, "wvn": wv_nat,
         "wout": wout_T, "b": b, "ident": ident}
        for i in range(N_CORES)
    ]
    kw = {}
    if trace:
        kw = {"trace": True, "tmpdir": tmpdir}
    res = run_bass_kernel_spmd(nc, in_maps, core_ids=list(range(N_CORES)), **kw)
    y = np.concatenate([res.results[i]["y"] for i in range(N_CORES)], axis=0)
    return y.reshape(B, DIM, 64, 64), res


def kernel(x, w_qkv, w_out, b_out):
    y, _ = _run(np.asarray(x), np.asarray(w_qkv), np.asarray(w_out),
                np.asarray(b_out))
    return y
